# revision 2
# baseline (speedup 1.0000x reference)
"""Trainium2 Bass kernel for nn_MixtureOfExperts (moe_routing).

Strategy (expert-parallel, per the sharding hint):
  - Host computes the tiny router (N x D @ D x E = 0.1% of total FLOPs),
    top-k selection and softmax combine weights in fp32 numpy.
  - Tokens are dispatched per expert on the host (gather), padded to a
    common capacity C (multiple of 128), and each of the 8 NeuronCores
    computes the SwiGLU expert MLP for ONE expert over its token batch:
        out_e = (x_e @ W_e) * silu(x_e @ V_e)        [C, H]
    Matmuls run in bf16 (fp32 PSUM accumulation), which is ~4x faster
    than fp32 on the PE array and well within tolerance.
  - Host scatter-adds the per-expert outputs weighted by the combine
    probabilities (exactly the reference's zeros+scatter-add semantics).

Device kernel layout per core:
  xT   [D, C]  bf16   x_e^T, resident in SBUF as [128, D/128, C]
  w/v  [H/512, D, 512] bf16  h-slab-major swizzled weights (streamed)
  out  [C, H] fp32
  Loop: for each h-slab (512 cols), for each token tile (128 tokens):
  16 accumulating matmuls per matrix (lhsT = stationary xT tile,
  rhs = weight slab slice), then ACT silu + DVE multiply, DMA out.
"""

import numpy as np
import ml_dtypes

P = 128
D = 2048
H = 2048
E = 8
N_CORES = 8
HSLAB = 512

_compiled = {}


def _build(C):
    """Build the Bass/Tile kernel for token capacity C (multiple of 128)."""
    import concourse.bacc as bacc
    import concourse.mybir as mybir
    import concourse.tile as tile

    KT = D // P          # 16 contraction tiles
    HT = H // HSLAB      # 4 h-slabs
    CT = C // P          # token tiles

    nc = bacc.Bacc("TRN2", target_bir_lowering=False, debug=False)
    bf16 = mybir.dt.bfloat16
    f32 = mybir.dt.float32

    xT = nc.dram_tensor("xt", [D, C], bf16, kind="ExternalInput").ap()
    Wd = nc.dram_tensor("w", [HT, D, HSLAB], bf16, kind="ExternalInput").ap()
    Vd = nc.dram_tensor("v", [HT, D, HSLAB], bf16, kind="ExternalInput").ap()
    out = nc.dram_tensor("out", [C, H], f32, kind="ExternalOutput").ap()

    with tile.TileContext(nc) as tc:
        with (
            tc.tile_pool(name="xt", bufs=1) as xt_pool,
            tc.tile_pool(name="wv", bufs=4) as wv_pool,
            tc.tile_pool(name="elt", bufs=6) as elt_pool,
            tc.tile_pool(name="psum", bufs=6, space="PSUM") as psum_pool,
        ):
            # Resident x^T: [128, KT, C]
            xt_sb = xt_pool.tile([P, KT, C], bf16)
            for d in range(KT):
                nc.sync.dma_start(xt_sb[:, d, :], xT[d * P:(d + 1) * P, :])

            for ih in range(HT):
                slabs = []
                for dram in (Wd, Vd):
                    sl = wv_pool.tile([P, KT, HSLAB], bf16, tag="wv")
                    for d in range(KT):
                        nc.sync.dma_start(sl[:, d, :], dram[ih, d * P:(d + 1) * P, :])
                    slabs.append(sl)
                wsl, vsl = slabs

                for c in range(CT):
                    pa = psum_pool.tile([P, HSLAB], f32, tag="ps")
                    for d in range(KT):
                        nc.tensor.matmul(
                            pa, xt_sb[:, d, c * P:(c + 1) * P], wsl[:, d, :],
                            start=(d == 0), stop=(d == KT - 1),
                        )
                    pb = psum_pool.tile([P, HSLAB], f32, tag="ps")
                    for d in range(KT):
                        nc.tensor.matmul(
                            pb, xt_sb[:, d, c * P:(c + 1) * P], vsl[:, d, :],
                            start=(d == 0), stop=(d == KT - 1),
                        )
                    sil = elt_pool.tile([P, HSLAB], f32, tag="sil")
                    nc.scalar.activation(sil, pb, mybir.ActivationFunctionType.Silu)
                    ot = elt_pool.tile([P, HSLAB], f32, tag="ot")
                    nc.vector.tensor_mul(ot, pa, sil)
                    nc.sync.dma_start(
                        out[c * P:(c + 1) * P, ih * HSLAB:(ih + 1) * HSLAB], ot
                    )
    nc.compile()
    return nc


def _get_kernel(C):
    if C not in _compiled:
        _compiled[C] = _build(C)
    return _compiled[C]


def _route(xf, router_w, router_b, k):
    """fp32 router: top-k ids and softmax combine weights, per expert."""
    logits = xf @ router_w.astype(np.float32) + router_b.astype(np.float32)
    order = np.argsort(-logits, axis=1)[:, :k]          # [N, k] expert ids
    top_logits = np.take_along_axis(logits, order, axis=1)
    m = top_logits.max(axis=1, keepdims=True)
    p = np.exp(top_logits - m)
    p /= p.sum(axis=1, keepdims=True)                   # [N, k]
    ids, wts = [], []
    for e in range(E):
        mask = order == e                               # [N, k]
        tok = np.nonzero(mask.any(axis=1))[0]
        wt = (p * mask).sum(axis=1)[tok]
        ids.append(tok)
        wts.append(wt.astype(np.float32))
    return ids, wts


def run(inputs, trace=False, trace_cores=None):
    """Full pipeline. Returns (output, BassKernelResults)."""
    from concourse.bass_utils import run_bass_kernel_spmd

    x = np.asarray(inputs["x"], dtype=np.float32)
    W = np.asarray(inputs["W"], dtype=np.float32)
    V = np.asarray(inputs["V"], dtype=np.float32)
    router_w = np.asarray(inputs["router_w"])
    router_b = np.asarray(inputs["router_b"])
    k = int(np.asarray(inputs["top_k"]))

    B, T, d = x.shape
    assert d == D and W.shape == (E, D, H) and V.shape == (E, D, H)
    N = B * T
    xf = x.reshape(N, D)

    ids, wts = _route(xf, router_w, router_b, k)
    C = max(max(len(i) for i in ids), 1)
    C = ((C + P - 1) // P) * P

    nc = _get_kernel(C)

    bf16 = ml_dtypes.bfloat16
    HT = H // HSLAB
    in_maps = []
    for e in range(E):
        ne = len(ids[e])
        xT_e = np.zeros((D, C), dtype=bf16)
        xT_e[:, :ne] = xf[ids[e]].T.astype(bf16)
        w_e = np.ascontiguousarray(
            W[e].reshape(D, HT, HSLAB).transpose(1, 0, 2)
        ).astype(bf16)
        v_e = np.ascontiguousarray(
            V[e].reshape(D, HT, HSLAB).transpose(1, 0, 2)
        ).astype(bf16)
        in_maps.append({"xt": xT_e, "w": w_e, "v": v_e})

    res = run_bass_kernel_spmd(
        nc,
        in_maps,
        core_ids=list(range(N_CORES)),
        trace=trace,
        trace_cores=trace_cores,
    )

    outf = np.zeros((N, H), dtype=np.float32)
    for e in range(E):
        ne = len(ids[e])
        if ne:
            y = res.results[e]["out"][:ne]              # [ne, H] fp32
            outf[ids[e]] += y * wts[e][:, None]
    return outf.reshape(B, T, H), res


def kernel(**inputs):
    out, _ = run(inputs, trace=False)
    return out


# revision 4
# speedup vs baseline: 1.0324x; 1.0324x over previous
"""Trainium2 Bass kernel for nn_MixtureOfExperts (moe_routing).

Strategy (expert-parallel, per the sharding hint):
  - Host computes the tiny router (N x D @ D x E = 0.1% of total FLOPs),
    top-k selection and softmax combine weights in fp32 numpy.
  - Tokens are dispatched per expert on the host (gather), padded to a
    common capacity C (multiple of 128), and each of the 8 NeuronCores
    computes the SwiGLU expert MLP for ONE expert over its token batch:
        out_e = (x_e @ W_e) * silu(x_e @ V_e)        [C, H]
    Matmuls run in bf16 (fp32 PSUM accumulation), which is ~4x faster
    than fp32 on the PE array and well within tolerance.
  - Host scatter-adds the per-expert outputs weighted by the combine
    probabilities (exactly the reference's zeros+scatter-add semantics).

Device kernel layout per core:
  xT   [D, C]  bf16   x_e^T, resident in SBUF as [128, D/128, C]
  w/v  [H/512, D, 512] bf16  h-slab-major swizzled weights (streamed)
  out  [C, H] fp32
  Loop: for each h-slab (512 cols), for each token tile (128 tokens):
  16 accumulating matmuls per matrix (lhsT = stationary xT tile,
  rhs = weight slab slice), then ACT silu + DVE multiply, DMA out.
"""

import numpy as np
import ml_dtypes

P = 128
D = 2048
H = 2048
E = 8
N_CORES = 8
HSLAB = 512

_compiled = {}


def _build(C):
    """Build the Bass/Tile kernel for token capacity C (multiple of 128)."""
    import concourse.bacc as bacc
    import concourse.mybir as mybir
    import concourse.tile as tile

    KT = D // P          # 16 contraction tiles
    HT = H // HSLAB      # 4 h-slabs
    CT = C // P          # token tiles

    nc = bacc.Bacc("TRN2", target_bir_lowering=False, debug=False)
    bf16 = mybir.dt.bfloat16
    f32 = mybir.dt.float32

    xT = nc.dram_tensor("xt", [D, C], bf16, kind="ExternalInput").ap()
    Wd = nc.dram_tensor("w", [HT, D, HSLAB], bf16, kind="ExternalInput").ap()
    Vd = nc.dram_tensor("v", [HT, D, HSLAB], bf16, kind="ExternalInput").ap()
    out = nc.dram_tensor("out", [C, H], f32, kind="ExternalOutput").ap()

    with tile.TileContext(nc) as tc:
        with (
            tc.tile_pool(name="xt", bufs=1) as xt_pool,
            tc.tile_pool(name="wv", bufs=6) as wv_pool,
            tc.tile_pool(name="elt", bufs=6) as elt_pool,
            tc.tile_pool(name="psum", bufs=6, space="PSUM") as psum_pool,
        ):
            # Resident x^T: [128, KT, C].  Interleave the ramp-critical
            # DMAs (xT, first W slab, first V slab) in d-order so the PE
            # can start the first accumulation chain as data arrives.
            xt_sb = xt_pool.tile([P, KT, C], bf16)
            w0 = wv_pool.tile([P, KT, HSLAB], bf16, tag="wv")
            v0 = wv_pool.tile([P, KT, HSLAB], bf16, tag="wv")
            for d in range(KT):
                nc.sync.dma_start(xt_sb[:, d, :], xT[d * P:(d + 1) * P, :])
                nc.sync.dma_start(w0[:, d, :], Wd[0, d * P:(d + 1) * P, :])
                nc.sync.dma_start(v0[:, d, :], Vd[0, d * P:(d + 1) * P, :])

            for ih in range(HT):
                if ih == 0:
                    wsl, vsl = w0, v0
                else:
                    # Steady-state slabs: one big DMA each (prefetched
                    # a couple of h-slabs ahead via the pool rotation).
                    wsl = wv_pool.tile([P, KT, HSLAB], bf16, tag="wv")
                    nc.sync.dma_start(
                        wsl, Wd[ih].rearrange("(ko p) n -> p ko n", p=P)
                    )
                    vsl = wv_pool.tile([P, KT, HSLAB], bf16, tag="wv")
                    nc.sync.dma_start(
                        vsl, Vd[ih].rearrange("(ko p) n -> p ko n", p=P)
                    )

                for c in range(CT):
                    pa = psum_pool.tile([P, HSLAB], f32, tag="ps")
                    for d in range(KT):
                        nc.tensor.matmul(
                            pa, xt_sb[:, d, c * P:(c + 1) * P], wsl[:, d, :],
                            start=(d == 0), stop=(d == KT - 1),
                        )
                    pb = psum_pool.tile([P, HSLAB], f32, tag="ps")
                    for d in range(KT):
                        nc.tensor.matmul(
                            pb, xt_sb[:, d, c * P:(c + 1) * P], vsl[:, d, :],
                            start=(d == 0), stop=(d == KT - 1),
                        )
                    sil = elt_pool.tile([P, HSLAB], f32, tag="sil")
                    nc.scalar.activation(sil, pb, mybir.ActivationFunctionType.Silu)
                    ot = elt_pool.tile([P, HSLAB], f32, tag="ot")
                    nc.vector.tensor_mul(ot, pa, sil)
                    # Output DMAs go on the (otherwise idle) GpSimd queue so
                    # the Sync queue stays dedicated to weight prefetch.
                    nc.gpsimd.dma_start(
                        out[c * P:(c + 1) * P, ih * HSLAB:(ih + 1) * HSLAB], ot
                    )
    nc.compile()
    return nc


def _get_kernel(C):
    if C not in _compiled:
        _compiled[C] = _build(C)
    return _compiled[C]


def _route(xf, router_w, router_b, k):
    """fp32 router: top-k ids and softmax combine weights, per expert."""
    logits = xf @ router_w.astype(np.float32) + router_b.astype(np.float32)
    order = np.argsort(-logits, axis=1)[:, :k]          # [N, k] expert ids
    top_logits = np.take_along_axis(logits, order, axis=1)
    m = top_logits.max(axis=1, keepdims=True)
    p = np.exp(top_logits - m)
    p /= p.sum(axis=1, keepdims=True)                   # [N, k]
    ids, wts = [], []
    for e in range(E):
        mask = order == e                               # [N, k]
        tok = np.nonzero(mask.any(axis=1))[0]
        wt = (p * mask).sum(axis=1)[tok]
        ids.append(tok)
        wts.append(wt.astype(np.float32))
    return ids, wts


def run(inputs, trace=False, trace_cores=None):
    """Full pipeline. Returns (output, BassKernelResults)."""
    from concourse.bass_utils import run_bass_kernel_spmd

    x = np.asarray(inputs["x"], dtype=np.float32)
    W = np.asarray(inputs["W"], dtype=np.float32)
    V = np.asarray(inputs["V"], dtype=np.float32)
    router_w = np.asarray(inputs["router_w"])
    router_b = np.asarray(inputs["router_b"])
    k = int(np.asarray(inputs["top_k"]))

    B, T, d = x.shape
    assert d == D and W.shape == (E, D, H) and V.shape == (E, D, H)
    N = B * T
    xf = x.reshape(N, D)

    ids, wts = _route(xf, router_w, router_b, k)
    C = max(max(len(i) for i in ids), 1)
    C = ((C + P - 1) // P) * P

    nc = _get_kernel(C)

    bf16 = ml_dtypes.bfloat16
    HT = H // HSLAB
    in_maps = []
    for e in range(E):
        ne = len(ids[e])
        xT_e = np.zeros((D, C), dtype=bf16)
        xT_e[:, :ne] = xf[ids[e]].T.astype(bf16)
        w_e = np.ascontiguousarray(
            W[e].reshape(D, HT, HSLAB).transpose(1, 0, 2)
        ).astype(bf16)
        v_e = np.ascontiguousarray(
            V[e].reshape(D, HT, HSLAB).transpose(1, 0, 2)
        ).astype(bf16)
        in_maps.append({"xt": xT_e, "w": w_e, "v": v_e})

    res = run_bass_kernel_spmd(
        nc,
        in_maps,
        core_ids=list(range(N_CORES)),
        trace=trace,
        trace_cores=trace_cores,
    )

    outf = np.zeros((N, H), dtype=np.float32)
    for e in range(E):
        ne = len(ids[e])
        if ne:
            y = res.results[e]["out"][:ne]              # [ne, H] fp32
            outf[ids[e]] += y * wts[e][:, None]
    return outf.reshape(B, T, H), res


def kernel(**inputs):
    out, _ = run(inputs, trace=False)
    return out


# revision 7
# speedup vs baseline: 1.0364x; 1.0039x over previous
"""Trainium2 Bass kernel for nn_MixtureOfExperts (moe_routing).

Strategy (expert-parallel + H-split, derived from the sharding hint):
  - Host computes the tiny router (N x D @ D x E = 0.1% of total FLOPs),
    top-k selection and softmax combine weights in fp32 numpy.
  - Experts are paired so their token-tile counts balance (a 9-tile
    expert with an 8-tile one).  Each pair is served by TWO cores, each
    computing one H-half (1024 cols) of BOTH experts of the pair:
        out_e = (x_e @ W_e) * silu(x_e @ V_e)
    This equalizes per-core matmul counts across all 8 cores at
    (T1+T2) token-tiles x (H/2) columns, below the max-expert padding
    cost of plain expert-parallel.  Matmuls run in bf16 (fp32 PSUM).
  - Host scatter-adds the per-expert outputs weighted by the combine
    probabilities (the reference's zeros+scatter-add semantics).

Device kernel layout per core (pair (a,b), h-half h):
  xta  [D, T1*128] bf16   tokens routed to expert a (transposed)
  xtb  [D, T2*128] bf16   tokens routed to expert b
  w,v  [2, 2, D, 512] bf16  h-slab-major weights: [slot, sub-slab, D, 512]
  out  [(T1+T2)*128, 1024] fp32
  Loop: per (expert-slot, 512-col sub-slab), per 128-token tile:
  16 accumulating matmuls per matrix (lhsT = stationary x^T tile,
  rhs = weight slab slice), then ACT silu + DVE multiply, DMA out.
"""

import numpy as np
import ml_dtypes

P = 128
D = 2048
H = 2048
E = 8
N_CORES = 8
HSLAB = 512
HHALF = 1024

_compiled = {}


def _build(T1, T2):
    """Build the Bass/Tile kernel for token-tile counts (T1, T2)."""
    import concourse.bacc as bacc
    import concourse.mybir as mybir
    import concourse.tile as tile

    KT = D // P              # 16 contraction tiles
    C1, C2 = T1 * P, T2 * P
    C = C1 + C2

    nc = bacc.Bacc("TRN2", target_bir_lowering=False, debug=False)
    bf16 = mybir.dt.bfloat16
    f32 = mybir.dt.float32

    xta = nc.dram_tensor("xta", [D, C1], bf16, kind="ExternalInput").ap()
    xtb = nc.dram_tensor("xtb", [D, C2], bf16, kind="ExternalInput").ap()
    Wd = nc.dram_tensor("w", [2, 2, D, HSLAB], bf16, kind="ExternalInput").ap()
    Vd = nc.dram_tensor("v", [2, 2, D, HSLAB], bf16, kind="ExternalInput").ap()
    out = nc.dram_tensor("out", [C, HHALF], f32, kind="ExternalOutput").ap()

    with tile.TileContext(nc) as tc:
        with (
            tc.tile_pool(name="xt", bufs=1) as xt_pool,
            tc.tile_pool(name="wv", bufs=5) as wv_pool,
            tc.tile_pool(name="elt", bufs=6) as elt_pool,
            tc.tile_pool(name="psum", bufs=6, space="PSUM") as psum_pool,
        ):
            # Resident x^T for both experts of the pair.  Interleave the
            # ramp-critical DMAs (xta, first W slab, first V slab) in
            # d-order so the PE starts the first accumulation chain as
            # data arrives; xtb streams later (needed after T1 tiles).
            xta_sb = xt_pool.tile([P, KT, C1], bf16, tag="xta")
            xtb_sb = xt_pool.tile([P, KT, C2], bf16, tag="xtb")
            w00 = wv_pool.tile([P, KT, HSLAB], bf16, tag="wv")
            v00 = wv_pool.tile([P, KT, HSLAB], bf16, tag="wv")
            for d in range(KT):
                nc.sync.dma_start(xta_sb[:, d, :], xta[d * P:(d + 1) * P, :])
                nc.sync.dma_start(w00[:, d, :], Wd[0, 0, d * P:(d + 1) * P, :])
                nc.sync.dma_start(v00[:, d, :], Vd[0, 0, d * P:(d + 1) * P, :])
            for d in range(KT):
                nc.sync.dma_start(xtb_sb[:, d, :], xtb[d * P:(d + 1) * P, :])

            # jobs: (expert-slot, sub-slab, xt tile, tiles, c-tile base)
            jobs = [
                (0, 0, xta_sb, T1, 0),
                (0, 1, xta_sb, T1, 0),
                (1, 0, xtb_sb, T2, T1),
                (1, 1, xtb_sb, T2, T1),
            ]
            for slot, hs, xt_sb, tiles, cbase in jobs:
                if slot == 0 and hs == 0:
                    wsl, vsl = w00, v00
                else:
                    # Steady-state slabs: one big DMA each (prefetched
                    # ahead via the pool rotation).
                    wsl = wv_pool.tile([P, KT, HSLAB], bf16, tag="wv")
                    nc.sync.dma_start(
                        wsl, Wd[slot, hs].rearrange("(ko p) n -> p ko n", p=P)
                    )
                    vsl = wv_pool.tile([P, KT, HSLAB], bf16, tag="wv")
                    nc.sync.dma_start(
                        vsl, Vd[slot, hs].rearrange("(ko p) n -> p ko n", p=P)
                    )

                def a_phase(ct):
                    pa = psum_pool.tile([P, HSLAB], f32, tag="ps")
                    for d in range(KT):
                        nc.tensor.matmul(
                            pa, xt_sb[:, d, ct * P:(ct + 1) * P], wsl[:, d, :],
                            start=(d == 0), stop=(d == KT - 1),
                        )
                    return pa

                def b_phase_and_out(ct, pa):
                    c = cbase + ct
                    pb = psum_pool.tile([P, HSLAB], f32, tag="ps")
                    for d in range(KT):
                        nc.tensor.matmul(
                            pb, xt_sb[:, d, ct * P:(ct + 1) * P], vsl[:, d, :],
                            start=(d == 0), stop=(d == KT - 1),
                        )
                    sil = elt_pool.tile([P, HSLAB], f32, tag="sil")
                    nc.scalar.activation(sil, pb, mybir.ActivationFunctionType.Silu)
                    ot = elt_pool.tile([P, HSLAB], f32, tag="ot")
                    nc.vector.tensor_mul(ot, pa, sil)
                    nc.sync.dma_start(
                        out[c * P:(c + 1) * P, hs * HSLAB:(hs + 1) * HSLAB], ot
                    )

                # In the first job the V slab races the PE up the ramp:
                # lag its phases a few tiles behind the W phases to give
                # the v00 DMA extra arrival slack.
                lag = 3 if (slot == 0 and hs == 0) else 0
                pending = []
                for ct in range(tiles):
                    pending.append((ct, a_phase(ct)))
                    if len(pending) > lag:
                        b_phase_and_out(*pending.pop(0))
                for ct, pa in pending:
                    b_phase_and_out(ct, pa)
    nc.compile()
    return nc


def _get_kernel(T1, T2):
    key = (T1, T2)
    if key not in _compiled:
        _compiled[key] = _build(T1, T2)
    return _compiled[key]


def _route(xf, router_w, router_b, k):
    """fp32 router: per-expert token ids and softmax combine weights."""
    logits = xf @ router_w.astype(np.float32) + router_b.astype(np.float32)
    order = np.argsort(-logits, axis=1)[:, :k]          # [N, k] expert ids
    top_logits = np.take_along_axis(logits, order, axis=1)
    m = top_logits.max(axis=1, keepdims=True)
    p = np.exp(top_logits - m)
    p /= p.sum(axis=1, keepdims=True)                   # [N, k]
    ids, wts = [], []
    for e in range(E):
        mask = order == e                               # [N, k]
        tok = np.nonzero(mask.any(axis=1))[0]
        wt = (p * mask).sum(axis=1)[tok]
        ids.append(tok)
        wts.append(wt.astype(np.float32))
    return ids, wts


def run(inputs, trace=False, trace_cores=None):
    """Full pipeline. Returns (output, BassKernelResults)."""
    from concourse.bass_utils import run_bass_kernel_spmd

    x = np.asarray(inputs["x"], dtype=np.float32)
    W = np.asarray(inputs["W"], dtype=np.float32)
    V = np.asarray(inputs["V"], dtype=np.float32)
    router_w = np.asarray(inputs["router_w"])
    router_b = np.asarray(inputs["router_b"])
    k = int(np.asarray(inputs["top_k"]))

    B, T, d = x.shape
    assert d == D and W.shape == (E, D, H) and V.shape == (E, D, H)
    N = B * T
    xf = x.reshape(N, D)

    ids, wts = _route(xf, router_w, router_b, k)
    tcounts = [max(1, -(-len(i) // P)) for i in ids]     # tiles per expert

    # Pair the largest-tile expert with the smallest, 2nd with 2nd-smallest...
    order = sorted(range(E), key=lambda e: -tcounts[e])
    pairs = [(order[i], order[E - 1 - i]) for i in range(E // 2)]
    T1 = max(tcounts[a] for a, _ in pairs)
    T2 = max(tcounts[b] for _, b in pairs)
    C1, C2 = T1 * P, T2 * P

    nc = _get_kernel(T1, T2)

    bf16 = ml_dtypes.bfloat16
    # Per-expert weight slabs [4, D, 512] in bf16, cast once.
    Wr = [np.ascontiguousarray(
        W[e].reshape(D, 4, HSLAB).transpose(1, 0, 2)).astype(bf16)
        for e in range(E)]
    Vr = [np.ascontiguousarray(
        V[e].reshape(D, 4, HSLAB).transpose(1, 0, 2)).astype(bf16)
        for e in range(E)]

    in_maps = []
    for a, b in pairs:
        na, nb = len(ids[a]), len(ids[b])
        xta = np.zeros((D, C1), dtype=bf16)
        xta[:, :na] = xf[ids[a]].T.astype(bf16)
        xtb = np.zeros((D, C2), dtype=bf16)
        xtb[:, :nb] = xf[ids[b]].T.astype(bf16)
        for h in range(2):
            w_core = np.stack([Wr[a][2 * h:2 * h + 2], Wr[b][2 * h:2 * h + 2]])
            v_core = np.stack([Vr[a][2 * h:2 * h + 2], Vr[b][2 * h:2 * h + 2]])
            in_maps.append({"xta": xta, "xtb": xtb, "w": w_core, "v": v_core})

    res = run_bass_kernel_spmd(
        nc,
        in_maps,
        core_ids=list(range(N_CORES)),
        trace=trace,
        trace_cores=trace_cores,
    )

    outf = np.zeros((N, H), dtype=np.float32)
    for p, (a, b) in enumerate(pairs):
        lo = res.results[2 * p]["out"]                  # [C, 1024] h-half 0
        hi = res.results[2 * p + 1]["out"]              # [C, 1024] h-half 1
        na, nb = len(ids[a]), len(ids[b])
        if na:
            y = np.concatenate([lo[:na], hi[:na]], axis=1)
            outf[ids[a]] += y * wts[a][:, None]
        if nb:
            y = np.concatenate([lo[C1:C1 + nb], hi[C1:C1 + nb]], axis=1)
            outf[ids[b]] += y * wts[b][:, None]
    return outf.reshape(B, T, H), res


def kernel(**inputs):
    out, _ = run(inputs, trace=False)
    return out


# revision 10
# speedup vs baseline: 1.0424x; 1.0057x over previous
"""Trainium2 Bass kernel for nn_MixtureOfExperts (moe_routing).

Strategy (expert-parallel + H-split, derived from the sharding hint):
  - Host computes the tiny router (N x D @ D x E = 0.1% of total FLOPs),
    top-k selection and softmax combine weights in fp32 numpy.
  - Experts are paired so their token-tile counts balance (a 9-tile
    expert with an 8-tile one).  Each pair is served by TWO cores, each
    computing one H-half (1024 cols) of BOTH experts of the pair:
        out_e = (x_e @ W_e) * silu(x_e @ V_e)
    This equalizes per-core matmul counts across all 8 cores at
    (T1+T2) token-tiles x (H/2) columns, below the max-expert padding
    cost of plain expert-parallel.  Matmuls run in bf16 (fp32 PSUM).
  - Host scatter-adds the per-expert outputs weighted by the combine
    probabilities (the reference's zeros+scatter-add semantics).

Device kernel layout per core (pair (a,b), h-half h):
  xta  [D, T1*128] bf16   tokens routed to expert a (transposed)
  xtb  [D, T2*128] bf16   tokens routed to expert b
  w,v  [2, 2, D, 512] bf16  h-slab-major weights: [slot, sub-slab, D, 512]
  out  [(T1+T2)*128, 1024] fp32
  Loop: per (expert-slot, 512-col sub-slab), per 128-token tile:
  16 accumulating matmuls per matrix (lhsT = stationary x^T tile,
  rhs = weight slab slice), then ACT silu + DVE multiply, DMA out.
"""

import numpy as np
import ml_dtypes

P = 128
D = 2048
H = 2048
E = 8
N_CORES = 8
HSLAB = 512
HHALF = 1024

_compiled = {}


def _build(T1, T2):
    """Build the Bass/Tile kernel for token-tile counts (T1, T2)."""
    import concourse.bacc as bacc
    import concourse.mybir as mybir
    import concourse.tile as tile

    KT = D // P              # 16 contraction tiles
    C1, C2 = T1 * P, T2 * P
    C = C1 + C2

    nc = bacc.Bacc("TRN2", target_bir_lowering=False, debug=False)
    bf16 = mybir.dt.bfloat16
    f32 = mybir.dt.float32

    # SBUF budget guard (KB/partition): resident x^T costs 4*(T1+T2),
    # each weight-slab buffer 16, elementwise pools ~24.  Shrink slab
    # prefetch depth if a skewed routing inflates the token capacity.
    wv_bufs = 5
    while 4 * (T1 + T2) + 16 * wv_bufs + 28 > 188 and wv_bufs > 2:
        wv_bufs -= 1

    xta = nc.dram_tensor("xta", [D, C1], bf16, kind="ExternalInput").ap()
    xtb = nc.dram_tensor("xtb", [D, C2], bf16, kind="ExternalInput").ap()
    Wd = nc.dram_tensor("w", [2, 2, D, HSLAB], bf16, kind="ExternalInput").ap()
    Vd = nc.dram_tensor("v", [2, 2, D, HSLAB], bf16, kind="ExternalInput").ap()
    out = nc.dram_tensor("out", [C, HHALF], f32, kind="ExternalOutput").ap()

    with tile.TileContext(nc) as tc:
        with (
            tc.tile_pool(name="xt", bufs=1) as xt_pool,
            tc.tile_pool(name="wv", bufs=wv_bufs) as wv_pool,
            tc.tile_pool(name="elt", bufs=6) as elt_pool,
            tc.tile_pool(name="psum", bufs=6, space="PSUM") as psum_pool,
        ):
            # Resident x^T for both experts of the pair.  Interleave the
            # ramp-critical DMAs (xta, first W slab, first V slab) in
            # d-order so the PE starts the first accumulation chain as
            # data arrives; xtb streams later (needed after T1 tiles).
            xta_sb = xt_pool.tile([P, KT, C1], bf16, tag="xta")
            xtb_sb = xt_pool.tile([P, KT, C2], bf16, tag="xtb")
            w00 = wv_pool.tile([P, KT, HSLAB], bf16, tag="wv")
            v00 = wv_pool.tile([P, KT, HSLAB], bf16, tag="wv")
            for d in range(KT):
                nc.sync.dma_start(xta_sb[:, d, :], xta[d * P:(d + 1) * P, :])
                nc.sync.dma_start(w00[:, d, :], Wd[0, 0, d * P:(d + 1) * P, :])
                nc.sync.dma_start(v00[:, d, :], Vd[0, 0, d * P:(d + 1) * P, :])
            for d in range(KT):
                nc.sync.dma_start(xtb_sb[:, d, :], xtb[d * P:(d + 1) * P, :])

            # jobs: (expert-slot, sub-slab, xt tile, tiles, c-tile base)
            jobs = [
                (0, 0, xta_sb, T1, 0),
                (0, 1, xta_sb, T1, 0),
                (1, 0, xtb_sb, T2, T1),
                (1, 1, xtb_sb, T2, T1),
            ]
            for slot, hs, xt_sb, tiles, cbase in jobs:
                if slot == 0 and hs == 0:
                    wsl, vsl = w00, v00
                else:
                    # Steady-state slabs: one big DMA each (prefetched
                    # ahead via the pool rotation).
                    wsl = wv_pool.tile([P, KT, HSLAB], bf16, tag="wv")
                    nc.sync.dma_start(
                        wsl, Wd[slot, hs].rearrange("(ko p) n -> p ko n", p=P)
                    )
                    vsl = wv_pool.tile([P, KT, HSLAB], bf16, tag="wv")
                    nc.sync.dma_start(
                        vsl, Vd[slot, hs].rearrange("(ko p) n -> p ko n", p=P)
                    )

                def a_phase(ct):
                    pa = psum_pool.tile([P, HSLAB], f32, tag="ps")
                    for d in range(KT):
                        nc.tensor.matmul(
                            pa, xt_sb[:, d, ct * P:(ct + 1) * P], wsl[:, d, :],
                            start=(d == 0), stop=(d == KT - 1),
                        )
                    return pa

                def b_phase_and_out(ct, pa):
                    c = cbase + ct
                    pb = psum_pool.tile([P, HSLAB], f32, tag="ps")
                    for d in range(KT):
                        nc.tensor.matmul(
                            pb, xt_sb[:, d, ct * P:(ct + 1) * P], vsl[:, d, :],
                            start=(d == 0), stop=(d == KT - 1),
                        )
                    sil = elt_pool.tile([P, HSLAB], f32, tag="sil")
                    nc.scalar.activation(sil, pb, mybir.ActivationFunctionType.Silu)
                    ot = elt_pool.tile([P, HSLAB], f32, tag="ot")
                    nc.vector.tensor_mul(ot, pa, sil)
                    nc.sync.dma_start(
                        out[c * P:(c + 1) * P, hs * HSLAB:(hs + 1) * HSLAB], ot
                    )

                # In the first job the V slab races the PE up the ramp:
                # lag its phases a few tiles behind the W phases to give
                # the v00 DMA extra arrival slack.
                lag = 3 if (slot == 0 and hs == 0) else 0
                pending = []
                for ct in range(tiles):
                    pending.append((ct, a_phase(ct)))
                    if len(pending) > lag:
                        b_phase_and_out(*pending.pop(0))
                for ct, pa in pending:
                    b_phase_and_out(ct, pa)
    nc.compile()
    return nc


def _get_kernel(T1, T2):
    key = (T1, T2)
    if key not in _compiled:
        _compiled[key] = _build(T1, T2)
    return _compiled[key]


def _route(xf, router_w, router_b, k):
    """fp32 router: per-expert token ids and softmax combine weights."""
    logits = xf @ router_w.astype(np.float32) + router_b.astype(np.float32)
    # stable: ties resolve to the lower expert index, like lax.top_k
    order = np.argsort(-logits, axis=1, kind="stable")[:, :k]   # [N, k]
    top_logits = np.take_along_axis(logits, order, axis=1)
    m = top_logits.max(axis=1, keepdims=True)
    p = np.exp(top_logits - m)
    p /= p.sum(axis=1, keepdims=True)                   # [N, k]
    ids, wts = [], []
    for e in range(E):
        mask = order == e                               # [N, k]
        tok = np.nonzero(mask.any(axis=1))[0]
        wt = (p * mask).sum(axis=1)[tok]
        ids.append(tok)
        wts.append(wt.astype(np.float32))
    return ids, wts


def run(inputs, trace=False, trace_cores=None):
    """Full pipeline. Returns (output, BassKernelResults)."""
    from concourse.bass_utils import run_bass_kernel_spmd

    x = np.asarray(inputs["x"], dtype=np.float32)
    W = np.asarray(inputs["W"], dtype=np.float32)
    V = np.asarray(inputs["V"], dtype=np.float32)
    router_w = np.asarray(inputs["router_w"])
    router_b = np.asarray(inputs["router_b"])
    k = int(np.asarray(inputs["top_k"]))

    B, T, d = x.shape
    assert d == D and W.shape == (E, D, H) and V.shape == (E, D, H)
    N = B * T
    xf = x.reshape(N, D)

    ids, wts = _route(xf, router_w, router_b, k)
    tcounts = [max(1, -(-len(i) // P)) for i in ids]     # tiles per expert

    # Pair the largest-tile expert with the smallest, 2nd with 2nd-smallest...
    order = sorted(range(E), key=lambda e: -tcounts[e])
    pairs = [(order[i], order[E - 1 - i]) for i in range(E // 2)]
    T1 = max(tcounts[a] for a, _ in pairs)
    T2 = max(tcounts[b] for _, b in pairs)
    C1, C2 = T1 * P, T2 * P

    nc = _get_kernel(T1, T2)

    bf16 = ml_dtypes.bfloat16
    # Per-expert weight slabs [4, D, 512] in bf16, cast once.
    Wr = [np.ascontiguousarray(
        W[e].reshape(D, 4, HSLAB).transpose(1, 0, 2)).astype(bf16)
        for e in range(E)]
    Vr = [np.ascontiguousarray(
        V[e].reshape(D, 4, HSLAB).transpose(1, 0, 2)).astype(bf16)
        for e in range(E)]

    in_maps = []
    for a, b in pairs:
        na, nb = len(ids[a]), len(ids[b])
        xta = np.zeros((D, C1), dtype=bf16)
        xta[:, :na] = xf[ids[a]].T.astype(bf16)
        xtb = np.zeros((D, C2), dtype=bf16)
        xtb[:, :nb] = xf[ids[b]].T.astype(bf16)
        for h in range(2):
            w_core = np.stack([Wr[a][2 * h:2 * h + 2], Wr[b][2 * h:2 * h + 2]])
            v_core = np.stack([Vr[a][2 * h:2 * h + 2], Vr[b][2 * h:2 * h + 2]])
            in_maps.append({"xta": xta, "xtb": xtb, "w": w_core, "v": v_core})

    res = run_bass_kernel_spmd(
        nc,
        in_maps,
        core_ids=list(range(N_CORES)),
        trace=trace,
        trace_cores=trace_cores,
    )

    outf = np.zeros((N, H), dtype=np.float32)
    for p, (a, b) in enumerate(pairs):
        lo = res.results[2 * p]["out"]                  # [C, 1024] h-half 0
        hi = res.results[2 * p + 1]["out"]              # [C, 1024] h-half 1
        na, nb = len(ids[a]), len(ids[b])
        if na:
            y = np.concatenate([lo[:na], hi[:na]], axis=1)
            outf[ids[a]] += y * wts[a][:, None]
        if nb:
            y = np.concatenate([lo[C1:C1 + nb], hi[C1:C1 + nb]], axis=1)
            outf[ids[b]] += y * wts[b][:, None]
    return outf.reshape(B, T, H), res


def kernel(**inputs):
    out, _ = run(inputs, trace=False)
    return out


# revision 14
# speedup vs baseline: 1.1293x; 1.0834x over previous
"""Trainium2 Bass kernel for nn_MixtureOfExperts (moe_routing).

Strategy (expert-parallel + H-split, derived from the sharding hint):
  - Host computes the tiny router (N x D @ D x E = 0.1% of total FLOPs),
    top-k selection and softmax combine weights in fp32 numpy.
  - Experts are paired so their token-tile counts balance (a 9-tile
    expert with an 8-tile one).  Each pair is served by TWO cores, each
    computing one H-half (1024 cols) of BOTH experts of the pair:
        out_e = (x_e @ W_e) * silu(x_e @ V_e)
    This equalizes per-core matmul counts across all 8 cores at
    (T1+T2) token-tiles x (H/2) columns, below the max-expert padding
    cost of plain expert-parallel.  Matmuls run in bf16 (fp32 PSUM).
  - Host scatter-adds the per-expert outputs weighted by the combine
    probabilities (the reference's zeros+scatter-add semantics).

Device kernel layout per core (pair (a,b), h-half h):
  xta  [D, T1*128] bf16   tokens routed to expert a (transposed)
  xtb  [D, T2*128] bf16   tokens routed to expert b
  w,v  [2, 2, D, 512] bf16  h-slab-major weights: [slot, sub-slab, D, 512]
  out  [(T1+T2)*128, 1024] fp32
  Loop: per (expert-slot, 512-col sub-slab), per 128-token tile:
  16 accumulating matmuls per matrix (lhsT = stationary x^T tile,
  rhs = weight slab slice), then ACT silu + DVE multiply, DMA out.
"""

import numpy as np
import ml_dtypes

P = 128
D = 2048
H = 2048
E = 8
N_CORES = 8
HSLAB = 512
HHALF = 1024

_compiled = {}


def _build(T1, T2):
    """Build the Bass/Tile kernel for token-tile counts (T1, T2)."""
    import concourse.bacc as bacc
    import concourse.mybir as mybir
    import concourse.tile as tile

    KT = D // P              # 16 contraction tiles
    C1, C2 = T1 * P, T2 * P
    C = C1 + C2

    nc = bacc.Bacc("TRN2", target_bir_lowering=False, debug=False)
    bf16 = mybir.dt.bfloat16
    f32 = mybir.dt.float32

    # SBUF budget guard (KB/partition): resident x^T costs 4*(T1+T2),
    # each weight-slab buffer 16, elementwise pools ~24.  Shrink slab
    # prefetch depth if a skewed routing inflates the token capacity.
    wv_bufs = 5
    while 4 * (T1 + T2) + 16 * wv_bufs + 28 > 188 and wv_bufs > 2:
        wv_bufs -= 1

    xta = nc.dram_tensor("xta", [D, C1], bf16, kind="ExternalInput").ap()
    xtb = nc.dram_tensor("xtb", [D, C2], bf16, kind="ExternalInput").ap()
    Wd = nc.dram_tensor("w", [2, 2, D, HSLAB], bf16, kind="ExternalInput").ap()
    Vd = nc.dram_tensor("v", [2, 2, D, HSLAB], bf16, kind="ExternalInput").ap()
    out = nc.dram_tensor("out", [C, HHALF], f32, kind="ExternalOutput").ap()

    with tile.TileContext(nc) as tc:
        with (
            tc.tile_pool(name="xt", bufs=1) as xt_pool,
            tc.tile_pool(name="wv", bufs=wv_bufs) as wv_pool,
            tc.tile_pool(name="elt", bufs=6) as elt_pool,
            tc.tile_pool(name="psum", bufs=8, space="PSUM") as psum_pool,
        ):
            # Resident x^T for both experts of the pair.  Interleave the
            # ramp-critical DMAs (xta, first W slab, first V slab) in
            # d-order so the PE starts the first accumulation chain as
            # data arrives; xtb streams later (needed after T1 tiles).
            xta_sb = xt_pool.tile([P, KT, C1], bf16, tag="xta")
            xtb_sb = xt_pool.tile([P, KT, C2], bf16, tag="xtb")
            w00 = wv_pool.tile([P, KT, HSLAB], bf16, tag="wv")
            v00 = wv_pool.tile([P, KT, HSLAB], bf16, tag="wv")
            for d in range(KT):
                nc.sync.dma_start(xta_sb[:, d, :], xta[d * P:(d + 1) * P, :])
                nc.sync.dma_start(w00[:, d, :], Wd[0, 0, d * P:(d + 1) * P, :])
                nc.sync.dma_start(v00[:, d, :], Vd[0, 0, d * P:(d + 1) * P, :])
            for d in range(KT):
                nc.sync.dma_start(xtb_sb[:, d, :], xtb[d * P:(d + 1) * P, :])

            # jobs: (expert-slot, sub-slab, xt tile, tiles, c-tile base)
            jobs = [
                (0, 0, xta_sb, T1, 0),
                (0, 1, xta_sb, T1, 0),
                (1, 0, xtb_sb, T2, T1),
                (1, 1, xtb_sb, T2, T1),
            ]
            for slot, hs, xt_sb, tiles, cbase in jobs:
                if slot == 0 and hs == 0:
                    wsl, vsl = w00, v00
                else:
                    # Steady-state slabs: one big DMA each (prefetched
                    # ahead via the pool rotation).
                    wsl = wv_pool.tile([P, KT, HSLAB], bf16, tag="wv")
                    nc.sync.dma_start(
                        wsl, Wd[slot, hs].rearrange("(ko p) n -> p ko n", p=P)
                    )
                    vsl = wv_pool.tile([P, KT, HSLAB], bf16, tag="wv")
                    nc.sync.dma_start(
                        vsl, Vd[slot, hs].rearrange("(ko p) n -> p ko n", p=P)
                    )

                def a_phase(ct):
                    pa = psum_pool.tile([P, HSLAB], f32, tag="ps")
                    for d in range(KT):
                        nc.tensor.matmul(
                            pa, xt_sb[:, d, ct * P:(ct + 1) * P], wsl[:, d, :],
                            start=(d == 0), stop=(d == KT - 1),
                        )
                    return pa

                def b_phase_and_out(ct, pa, split=False):
                    c = cbase + ct
                    # For the very last tile, run the gate matmuls in two
                    # half-width accumulation groups so the SiLU/mul/DMA of
                    # half 0 overlaps the matmuls of half 1 (shortens the
                    # kernel tail by ~1us).
                    halves = (
                        [(0, HSLAB // 2), (HSLAB // 2, HSLAB)] if split
                        else [(0, HSLAB)]
                    )
                    for h0, h1 in halves:
                        nh = h1 - h0
                        pb = psum_pool.tile([P, HSLAB], f32, tag="ps")
                        for d in range(KT):
                            nc.tensor.matmul(
                                pb[:, :nh],
                                xt_sb[:, d, ct * P:(ct + 1) * P],
                                vsl[:, d, h0:h1],
                                start=(d == 0), stop=(d == KT - 1),
                            )
                        sil = elt_pool.tile([P, HSLAB], f32, tag="sil")
                        nc.scalar.activation(
                            sil[:, :nh], pb[:, :nh],
                            mybir.ActivationFunctionType.Silu,
                        )
                        ot = elt_pool.tile([P, HSLAB], f32, tag="ot")
                        nc.vector.tensor_mul(ot[:, :nh], pa[:, h0:h1], sil[:, :nh])
                        nc.sync.dma_start(
                            out[c * P:(c + 1) * P,
                                hs * HSLAB + h0:hs * HSLAB + h1],
                            ot[:, :nh],
                        )

                # In the first job the V slab races the PE up the ramp:
                # lag its phases a few tiles behind the W phases to give
                # the v00 DMA extra arrival slack.
                lag = 3 if (slot == 0 and hs == 0) else 0
                last_ct = tiles - 1 if (slot == 1 and hs == 1) else -1
                pending = []
                for ct in range(tiles):
                    pending.append((ct, a_phase(ct)))
                    if len(pending) > lag:
                        pct, ppa = pending.pop(0)
                        b_phase_and_out(pct, ppa, split=(pct == last_ct))
                for pct, ppa in pending:
                    b_phase_and_out(pct, ppa, split=(pct == last_ct))
    nc.compile()
    return nc


def _get_kernel(T1, T2):
    key = (T1, T2)
    if key not in _compiled:
        _compiled[key] = _build(T1, T2)
    return _compiled[key]


def _route(xf, router_w, router_b, k):
    """fp32 router: per-expert token ids and softmax combine weights."""
    logits = xf @ router_w.astype(np.float32) + router_b.astype(np.float32)
    # stable: ties resolve to the lower expert index, like lax.top_k
    order = np.argsort(-logits, axis=1, kind="stable")[:, :k]   # [N, k]
    top_logits = np.take_along_axis(logits, order, axis=1)
    m = top_logits.max(axis=1, keepdims=True)
    p = np.exp(top_logits - m)
    p /= p.sum(axis=1, keepdims=True)                   # [N, k]
    ids, wts = [], []
    for e in range(E):
        mask = order == e                               # [N, k]
        tok = np.nonzero(mask.any(axis=1))[0]
        wt = (p * mask).sum(axis=1)[tok]
        ids.append(tok)
        wts.append(wt.astype(np.float32))
    return ids, wts


def run(inputs, trace=False, trace_cores=None):
    """Full pipeline. Returns (output, BassKernelResults)."""
    from concourse.bass_utils import run_bass_kernel_spmd

    x = np.asarray(inputs["x"], dtype=np.float32)
    W = np.asarray(inputs["W"], dtype=np.float32)
    V = np.asarray(inputs["V"], dtype=np.float32)
    router_w = np.asarray(inputs["router_w"])
    router_b = np.asarray(inputs["router_b"])
    k = int(np.asarray(inputs["top_k"]))

    B, T, d = x.shape
    assert d == D and W.shape == (E, D, H) and V.shape == (E, D, H)
    N = B * T
    xf = x.reshape(N, D)

    ids, wts = _route(xf, router_w, router_b, k)
    tcounts = [max(1, -(-len(i) // P)) for i in ids]     # tiles per expert

    # Pair the largest-tile expert with the smallest, 2nd with 2nd-smallest...
    order = sorted(range(E), key=lambda e: -tcounts[e])
    pairs = [(order[i], order[E - 1 - i]) for i in range(E // 2)]
    T1 = max(tcounts[a] for a, _ in pairs)
    T2 = max(tcounts[b] for _, b in pairs)
    C1, C2 = T1 * P, T2 * P

    nc = _get_kernel(T1, T2)

    bf16 = ml_dtypes.bfloat16
    # Per-expert weight slabs [4, D, 512] in bf16, cast once.
    Wr = [np.ascontiguousarray(
        W[e].reshape(D, 4, HSLAB).transpose(1, 0, 2)).astype(bf16)
        for e in range(E)]
    Vr = [np.ascontiguousarray(
        V[e].reshape(D, 4, HSLAB).transpose(1, 0, 2)).astype(bf16)
        for e in range(E)]

    in_maps = []
    for a, b in pairs:
        na, nb = len(ids[a]), len(ids[b])
        xta = np.zeros((D, C1), dtype=bf16)
        xta[:, :na] = xf[ids[a]].T.astype(bf16)
        xtb = np.zeros((D, C2), dtype=bf16)
        xtb[:, :nb] = xf[ids[b]].T.astype(bf16)
        for h in range(2):
            w_core = np.stack([Wr[a][2 * h:2 * h + 2], Wr[b][2 * h:2 * h + 2]])
            v_core = np.stack([Vr[a][2 * h:2 * h + 2], Vr[b][2 * h:2 * h + 2]])
            in_maps.append({"xta": xta, "xtb": xtb, "w": w_core, "v": v_core})

    res = run_bass_kernel_spmd(
        nc,
        in_maps,
        core_ids=list(range(N_CORES)),
        trace=trace,
        trace_cores=trace_cores,
    )

    outf = np.zeros((N, H), dtype=np.float32)
    for p, (a, b) in enumerate(pairs):
        lo = res.results[2 * p]["out"]                  # [C, 1024] h-half 0
        hi = res.results[2 * p + 1]["out"]              # [C, 1024] h-half 1
        na, nb = len(ids[a]), len(ids[b])
        if na:
            y = np.concatenate([lo[:na], hi[:na]], axis=1)
            outf[ids[a]] += y * wts[a][:, None]
        if nb:
            y = np.concatenate([lo[C1:C1 + nb], hi[C1:C1 + nb]], axis=1)
            outf[ids[b]] += y * wts[b][:, None]
    return outf.reshape(B, T, H), res


def kernel(**inputs):
    out, _ = run(inputs, trace=False)
    return out
